# revision 2
# baseline (speedup 1.0000x reference)
"""2-layer GCN (gather/scatter message passing) on 8 trn2 NeuronCores — v2.

Strategy: nodes (and incoming edges) partitioned across 8 cores by dst
range. Each core computes x@W1 (bf16) for its slice; slices exchanged
via bf16 AllGather (split in halves to overlap with compute). Groups =
src octants: gpsimd core g gathers from the table of octant g (= core
g's h slice, [16 feats x 12800 nodes] f32, upcast from the bf16
AllGather), no table replication. Edge streams per (core, group) are
split by src half and dst chunk-half into 4 quarters, processed in
blocks of 4 dst-chunks (~5.6k idx) so each ap_gather amortizes its
table scan. Messages (DVE mult by bf16 weights, in place), prefix-sum
per block (DVE scan, in place), per-slot segment sums extracted by a
boundary ap_gather + diff; src-half partials accumulated via two
accumulating PE matmuls against a 0/1 selection matrix. Layer-2
log_softmax runs Exp/Ln in batched sweeps to avoid activation-table
thrash.
"""
import sys, os
sys.path.insert(0, '/opt/trn_rl_repo')

import numpy as np
import ml_dtypes

# ---- problem constants (hardcoded per contract) ----
N_NODES = 100000
N_EDGES = 6400000
D_IN, D_F = 512, 16
NC = 8                   # cores
NPC_REAL = 12500         # real nodes per core
DPC = 320                # dst slots per chunk
NCHUNK = 40              # chunks per core
NPC = DPC * NCHUNK       # padded nodes per core (12800)
NPAD = NPC * NC          # padded global nodes (102400)
NGROUP = 8               # 16-partition groups per core = src octants
HALF = NPC // 2          # src half size (6400)
NSH = 2                  # src halves
NCH = 2                  # dst chunk halves
CPH = NCHUNK // NCH      # chunks per dst half (20)
BLK = 4                  # dst chunks per gather block
NBLK = CPH // BLK        # blocks per quarter (5)
BND = 1 + BLK * DPC      # boundary entries per block (1281)
BNDP = ((BND + 15) // 16) * 16  # padded to 1296
PAC = 400                # phase-A column chunk (16 per half)
NPA = NPC // PAC         # 32 phase-A chunks

bf16 = ml_dtypes.bfloat16


def _pad_id(n):
    return (n // NPC_REAL) * NPC + (n % NPC_REAL)


def _host_prep(x, edge_index, edge_weight):
    """Per-core inputs + the shared per-(srchalf, chunk) length schedule."""
    src = np.asarray(edge_index[0], dtype=np.int64)
    dst = np.asarray(edge_index[1], dtype=np.int64)
    w = np.asarray(edge_weight, dtype=np.float32)

    spad = _pad_id(src)
    grp = spad // NPC                        # src octant 0..7
    loc = spad % NPC
    sh = loc // HALF                         # src half 0/1
    sidx = (loc % HALF).astype(np.int16)     # 0..6399
    dcore = dst // NPC_REAL
    dloc = dst % NPC_REAL
    chunk = dloc // DPC                      # 0..39
    dslot = dloc % DPC                       # 0..319

    order = np.lexsort((dslot, chunk, sh, grp, dcore))
    so_sidx = sidx[order]; so_w = w[order].astype(bf16)
    so_core = dcore[order]; so_grp = grp[order]
    so_sh = sh[order]; so_chunk = chunk[order]; so_slot = dslot[order]

    # counts per (core, group, srchalf, chunk)
    cgsk = ((so_core * NGROUP + so_grp) * NSH + so_sh) * NCHUNK + so_chunk
    counts = np.bincount(cgsk, minlength=NC * NGROUP * NSH * NCHUNK)
    counts4 = counts.reshape(NC, NGROUP, NSH, NCHUNK)
    # schedule per (srchalf, chunk): 1 pad + max count, rounded to 64
    C = 1 + counts4.max(axis=(0, 1))         # [NSH, NCHUNK]
    C = ((C + 63) // 64) * 64
    C_off = np.concatenate([np.zeros((NSH, 1), np.int64), np.cumsum(C, axis=1)], axis=1)
    TOT = C_off[:, -1].astype(int)           # per src half

    # within-(core,group,sh,chunk) index of each sorted edge
    seg_off = np.concatenate([[0], np.cumsum(counts)])
    within = np.arange(N_EDGES) - seg_off[cgsk]

    idx_wr = []
    w_rep = []
    for S in range(NSH):
        m = so_sh == S
        TS = int(TOT[S])
        idx_all = np.zeros((NC, NGROUP, TS), dtype=np.int16)
        w_all = np.zeros((NC, NGROUP, TS), dtype=bf16)
        pos = C_off[S, so_chunk[m]] + 1 + within[m]
        lin = (so_core[m] * NGROUP + so_grp[m]) * TS + pos
        idx_all.reshape(-1)[lin] = so_sidx[m]
        w_all.reshape(-1)[lin] = so_w[m]
        # wrap idx into 16 partitions: part 16g+j holds list[j::16]
        iw = np.ascontiguousarray(
            idx_all.reshape(NC, NGROUP, TS // 16, 16).transpose(0, 1, 3, 2)
        ).reshape(NC, 128, TS // 16)
        idx_wr.append(iw)
        w_rep.append(np.repeat(w_all, 16, axis=1))

    # boundary position lists per (srchalf, block): [0] + 4 chunks x DPC ends
    cgsks = (((so_core * NGROUP + so_grp) * NSH + so_sh) * NCHUNK + so_chunk) * DPC + so_slot
    slot_counts = np.bincount(cgsks, minlength=NC * NGROUP * NSH * NCHUNK * DPC)
    slot_counts = slot_counts.reshape(NC, NGROUP, NSH, NCHUNK, DPC)
    bpos = np.cumsum(slot_counts, axis=4)    # end-of-slot counts

    bidx_wr = []
    for S in range(NSH):
        nblk_tot = NCH * NBLK                # 10 blocks
        blists = np.zeros((NC, NGROUP, nblk_tot, BNDP), dtype=np.int16)
        for H in range(NCH):
            for b in range(NBLK):
                k0 = H * CPH + b * BLK
                boff = C_off[S, k0]
                parts = [np.zeros((NC, NGROUP, 1), np.int64)]  # block-start pad
                for k in range(k0, k0 + BLK):
                    parts.append(C_off[S, k] - boff + bpos[:, :, S, k, :])
                lst = np.concatenate(parts, axis=2)            # [NC, NG, 1281]
                pad = np.repeat(lst[:, :, -1:], BNDP - BND, axis=2)
                blists[:, :, H * NBLK + b, :] = np.concatenate([lst, pad], axis=2)
        bw = np.ascontiguousarray(
            blists.reshape(NC, NGROUP, nblk_tot, BNDP // 16, 16)
            .transpose(0, 1, 4, 2, 3)
        ).reshape(NC, 128, nblk_tot * (BNDP // 16))
        bidx_wr.append(bw)

    # xT slices as [128, 4, NPC] bf16 (partition p holds rows p, 128+p,
    # 256+p, 384+p of x^T) so each phase-A chunk loads with ONE DMA
    xT = np.zeros((NC, 128, 4, NPC), dtype=bf16)
    xf = np.asarray(x, dtype=np.float32)
    for c in range(NC):
        xt = xf[c * NPC_REAL:(c + 1) * NPC_REAL, :].T.astype(bf16)  # [512, 12500]
        xT[c, :, :, :NPC_REAL] = xt.reshape(4, 128, NPC_REAL).transpose(1, 0, 2)

    return idx_wr, w_rep, bidx_wr, xT, C, TOT


def _build_program(C, TOT):
    import concourse.bass as bass
    import concourse.bacc as bacc
    import concourse.mybir as mybir
    from concourse.tile import TileContext

    f32 = mybir.dt.float32
    bf = mybir.dt.bfloat16
    i16 = mybir.dt.int16
    AO = mybir.AluOpType
    AF = mybir.ActivationFunctionType
    C_off = np.concatenate([np.zeros((NSH, 1), np.int64), np.cumsum(C, axis=1)], axis=1)

    nc = bacc.Bacc("TRN2", target_bir_lowering=False, debug=False, num_devices=NC)
    from concourse.bass import BassGpSimd

    def collective_on(eng, **kw):
        """All collectives on gpsimd (the only engine neuronxcc accepts
        here). Issue positions are chosen so the Pool-queue park during the
        transfer lands where DVE has a backlog to drain."""
        return nc.gpsimd.collective_compute(
            "AllGather", mybir.AluOpType.bypass,
            replica_groups=[list(range(NC))], **kw)

    # inputs
    xT_d = nc.dram_tensor("xT", [128, 4, NPC], bf, kind="ExternalInput")
    idx_d = [nc.dram_tensor(f"idx{S}", [128, int(TOT[S]) // 16], i16, kind="ExternalInput")
             for S in range(NSH)]
    w_d = [nc.dram_tensor(f"w{S}", [128, int(TOT[S])], bf, kind="ExternalInput")
           for S in range(NSH)]
    bidx_d = [nc.dram_tensor(f"bidx{S}", [128, NCH * NBLK * (BNDP // 16)], i16,
                             kind="ExternalInput") for S in range(NSH)]
    W1_d = nc.dram_tensor("W1", [D_IN, D_F], bf, kind="ExternalInput")
    W2_d = nc.dram_tensor("W2", [D_F, D_F], f32, kind="ExternalInput")
    b1_d = nc.dram_tensor("b1", [D_F, 1], f32, kind="ExternalInput")
    b2_d = nc.dram_tensor("b2", [D_F, 1], f32, kind="ExternalInput")
    sel_d = nc.dram_tensor("sel", [128, D_F], f32, kind="ExternalInput")
    id16_d = nc.dram_tensor("id16", [D_F, D_F], f32, kind="ExternalInput")
    out_d = nc.dram_tensor("out", [NPC, D_F], f32, kind="ExternalOutput")

    # internal DRAM: per-half h slices + allgathered tables (bf16)
    t1s = [nc.dram_tensor(f"t1s{S}", [D_F, HALF], bf) for S in range(NSH)]
    t2s = [nc.dram_tensor(f"t2s{S}", [D_F, HALF], bf) for S in range(NSH)]
    t1f = [nc.dram_tensor(f"t1f{S}", [NC * D_F, HALF], bf, addr_space="Shared")
           for S in range(NSH)]
    t2f = [nc.dram_tensor(f"t2f{S}", [NC * D_F, HALF], bf, addr_space="Shared")
           for S in range(NSH)]

    with TileContext(nc) as tc:
        with (tc.tile_pool(name="const", bufs=1) as cpool,
              tc.tile_pool(name="pA", bufs=4) as pa,
              tc.tile_pool(name="pAp", bufs=2, space="PSUM") as pap,
              tc.tile_pool(name="tblp", bufs=1) as tp,
              tc.tile_pool(name="tbfp", bufs=2) as tbp,
              tc.tile_pool(name="ed", bufs=2) as ep,
              tc.tile_pool(name="dva", bufs=1) as dap,
              tc.tile_pool(name="zs", bufs=1) as zp,
              tc.tile_pool(name="l2", bufs=2) as l2p,
              tc.tile_pool(name="aggp", bufs=2, space="PSUM") as ap_,
              tc.tile_pool(name="psz", bufs=2, space="PSUM") as pzp,
              tc.tile_pool(name="pst", bufs=2, space="PSUM") as ptp):
            w1t = cpool.tile([128, 4, D_F], bf)
            for kp in range(4):
                nc.sync.dma_start(out=w1t[:, kp, :], in_=W1_d[kp * 128:(kp + 1) * 128, :])
            w2t = cpool.tile([D_F, D_F], f32)
            nc.sync.dma_start(out=w2t[:], in_=W2_d[:])
            b1t = cpool.tile([D_F, 1], f32)
            nc.sync.dma_start(out=b1t[:], in_=b1_d[:])
            b2t = cpool.tile([D_F, 1], f32)
            nc.sync.dma_start(out=b2t[:], in_=b2_d[:])
            selt = cpool.tile([128, D_F], f32)
            nc.sync.dma_start(out=selt[:], in_=sel_d[:])
            id16t = cpool.tile([D_F, D_F], f32)
            nc.sync.dma_start(out=id16t[:], in_=id16_d[:])

            # ---------------- phase A: t1 = W1^T @ xT (bf16) --------------
            for ca in range(NPA):
                c0 = ca * PAC
                ps = pap.tile([D_F, PAC], f32, tag="t1ps")
                xtw = pa.tile([128, 4, PAC], bf, tag="xt")
                nc.sync.dma_start(out=xtw[:], in_=xT_d[:, :, c0:c0 + PAC])
                for kp in range(4):
                    nc.tensor.matmul(ps[:], lhsT=w1t[:, kp, :], rhs=xtw[:, kp, :],
                                     start=(kp == 0), stop=(kp == 3))
                if ca % 2 == 0:
                    t1c = pa.tile([D_F, 2, PAC], bf, tag="t1c", bufs=2)
                nc.vector.tensor_copy(t1c[:, ca % 2, :], ps[:])
                if ca % 2 == 1:
                    S = c0 // HALF
                    cs = (ca - 1) * PAC - S * HALF
                    nc.sync.dma_start(out=t1s[S][:, cs:cs + 2 * PAC], in_=t1c[:])
                if ca == NPA // 2 - 1:
                    collective_on(nc.sync, ins=[t1s[0][:]], outs=[t1f[0][:]])
            if True:
                tblh = [tp.tile([128, HALF], f32, name=f"tblh{S}", tag=f"tbl{S}")
                        for S in range(NSH)]
                dva = dap.tile([128, CPH * DPC], f32)
                zsave = zp.tile([128, 3 * CPH, D_F], f32)
                smsave = zp.tile([128, 3 * CPH], f32)

                def load_table(tf, S):
                    """upcast bf16 allgathered half into tblh[S].

                    The staging DMA is issued from the Act queue: it waits on
                    the AllGather, and SP's in-order queue must not head-block
                    the edge-stream prefetch DMAs behind it."""
                    for piece in range(2):
                        pc0 = piece * (HALF // 2)
                        tbf = tbp.tile([128, HALF // 2], bf, tag="tbf")
                        nc.scalar.dma_start(out=tbf[:], in_=tf[:, pc0:pc0 + HALF // 2])
                        nc.scalar.activation(tblh[S][:, pc0:pc0 + HALF // 2],
                                             tbf[:], AF.Copy)

                def block_front(S, H, b):
                    """loads + gather + mult + scan for one block"""
                    k0 = H * CPH + b * BLK
                    o0 = int(C_off[S, k0]); o1 = int(C_off[S, k0 + BLK])
                    B = o1 - o0
                    idxt = ep.tile([128, B // 16], i16, tag="idxt")
                    nc.sync.dma_start(out=idxt[:], in_=idx_d[S][:, o0 // 16:o1 // 16])
                    wt = ep.tile([128, B], bf, tag="wt")
                    nc.sync.dma_start(out=wt[:], in_=w_d[S][:, o0:o1])
                    bb = (H * NBLK + b) * (BNDP // 16)
                    bit = ep.tile([128, BNDP // 16], i16, tag="bit")
                    nc.sync.dma_start(out=bit[:], in_=bidx_d[S][:, bb:bb + BNDP // 16])

                    gt = ep.tile([128, B], f32, tag="gt")
                    nc.gpsimd.ap_gather(gt[:], tblh[S][:], idxt[:], channels=128,
                                        num_elems=HALF, d=1, num_idxs=B)
                    nc.vector.tensor_tensor(out=gt[:], in0=gt[:], in1=wt[:], op=AO.mult)
                    nc.vector.tensor_tensor_scan(gt[:], gt[:], gt[:], 0.0,
                                                 AO.add, AO.bypass)
                    return gt, bit, B

                def block_back(st, S, H, b):
                    """boundary gather + diff (Pool) for a finished block"""
                    gt, bit, B = st
                    bv = ep.tile([128, BNDP], f32, tag="bv")
                    nc.gpsimd.ap_gather(bv[:], gt[:], bit[:], channels=128,
                                        num_elems=B, d=1, num_idxs=BNDP)
                    dcols = BLK * DPC
                    if S == 0:
                        nc.gpsimd.tensor_tensor(
                            out=dva[:, b * dcols:(b + 1) * dcols],
                            in0=bv[:, 1:BND], in1=bv[:, 0:BND - 1], op=AO.subtract)
                    else:
                        dv = ep.tile([128, dcols], f32, tag="dv")
                        nc.gpsimd.tensor_tensor(
                            out=dv[:], in0=bv[:, 1:BND], in1=bv[:, 0:BND - 1],
                            op=AO.subtract)
                        nc.gpsimd.tensor_tensor(
                            out=dva[:, b * dcols:(b + 1) * dcols],
                            in0=dva[:, b * dcols:(b + 1) * dcols],
                            in1=dv[:], op=AO.add)

                def epi_chunk(kk, H, layer):
                    agg = ap_.tile([D_F, DPC], f32, tag="agg")
                    nc.tensor.matmul(agg[:], lhsT=selt[:],
                                     rhs=dva[:, kk * DPC:(kk + 1) * DPC],
                                     start=True, stop=True)
                    if layer == 1:
                        hc = ep.tile([D_F, DPC], bf, tag="hc")
                        nc.scalar.activation(hc[:], agg[:], AF.Relu, bias=b1t[:])
                        k = H * CPH + kk
                        Sd = k * DPC // HALF
                        cd = k * DPC - Sd * HALF
                        nc.sync.dma_start(out=t2s[Sd][:, cd:cd + DPC], in_=hc[:])
                    else:
                        asb = l2p.tile([D_F, DPC], f32, tag="asb", bufs=3)
                        nc.scalar.activation(asb[:], agg[:], AF.Copy)
                        zps = pzp.tile([D_F, DPC], f32, tag="zps")
                        nc.tensor.matmul(zps[:], lhsT=w2t[:], rhs=asb[:],
                                         start=True, stop=True)
                        zsb = l2p.tile([D_F, DPC], f32, tag="zsb", bufs=3)
                        nc.scalar.activation(zsb[:], zps[:], AF.Identity, bias=b2t[:])
                        for ji, j0 in enumerate(range(0, DPC, 128)):
                            bw = min(128, DPC - j0)
                            t = 3 * kk + ji
                            tps = ptp.tile([128, D_F], f32, tag="tps")
                            nc.tensor.transpose(tps[:bw, :], zsb[:, j0:j0 + bw],
                                                id16t[:])
                            mx = l2p.tile([128, 1], f32, tag="mx", bufs=4)
                            nc.vector.reduce_max(mx[:bw, :], tps[:bw, :],
                                                 axis=mybir.AxisListType.X)
                            nc.vector.tensor_scalar(out=zsave[:bw, t, :],
                                                    in0=tps[:bw, :],
                                                    scalar1=mx[:bw, :], scalar2=None,
                                                    op0=AO.subtract)
                            ez = l2p.tile([128, D_F], f32, tag="ez", bufs=4)
                            nc.scalar.activation(ez[:bw, :], zsave[:bw, t, :], AF.Exp,
                                                 accum_out=smsave[:bw, t:t + 1])

                def final_sweep(H):
                    ls = zp.tile([128, 3 * CPH], f32, tag="ls")
                    nc.scalar.activation(ls[:], smsave[:], AF.Ln)
                    for kk in range(CPH):
                        for ji, j0 in enumerate(range(0, DPC, 128)):
                            bw = min(128, DPC - j0)
                            t = 3 * kk + ji
                            ot = l2p.tile([128, D_F], f32, tag="ot", bufs=8)
                            nc.vector.tensor_scalar(out=ot[:bw, :],
                                                    in0=zsave[:bw, t, :],
                                                    scalar1=ls[:bw, t:t + 1],
                                                    scalar2=None,
                                                    op0=AO.subtract)
                            r0 = (H * CPH + kk) * DPC + j0
                            nc.sync.dma_start(out=out_d[r0:r0 + bw, :],
                                              in_=ot[:bw, :])

                def aggregate(tfs, layer, next_tfs=None, skip_s0_load=False,
                              entry_cc=None, mid_cc=None):
                    if not skip_s0_load:
                        load_table(tfs[0], 0)
                    if layer == 2:
                        nc.vector.memset(smsave[:], 1.0)
                    blocks = [(H, S, b) for H in range(NCH) for S in range(NSH)
                              for b in range(NBLK)]

                    def post_back(H, S, b):
                        """work enabled by block (H,S,b)'s dva contribution"""
                        if S != 1:
                            return
                        for kk in range(b * BLK, (b + 1) * BLK):
                            epi_chunk(kk, H, layer)
                        if b == NBLK - 1 and layer == 2:
                            final_sweep(H)
                            if H == 0:
                                nc.vector.memset(smsave[:], 1.0)

                    prev = None
                    for (H, S, b) in blocks:
                        if (H, S, b) == (0, 1, 0):
                            if entry_cc is not None:
                                collective_on(nc.gpsimd, ins=[entry_cc[0][:]],
                                              outs=[entry_cc[1][:]])
                            load_table(tfs[1], 1)
                        if (H, S, b) == (1, 1, 0):
                            if mid_cc is not None:
                                collective_on(nc.gpsimd, ins=[mid_cc[0][:]],
                                              outs=[mid_cc[1][:]])
                            if next_tfs is not None:
                                # prefetch next pass's S0 table (waits the
                                # mid collective + WAR on this pass's S0 reads)
                                load_table(next_tfs[0], 0)
                        st = block_front(S, H, b)
                        if prev is not None:
                            block_back(st_prev, *reversed_args(prev))
                            post_back(*prev)
                        prev = (H, S, b); st_prev = st
                    block_back(st_prev, *reversed_args(prev))
                    post_back(*prev)

                def reversed_args(hsb):
                    H, S, b = hsb
                    return (S, H, b)

                aggregate(t1f, 1, next_tfs=t2f,
                          entry_cc=(t1s[1], t1f[1]), mid_cc=(t2s[0], t2f[0]))
                aggregate(t2f, 2, skip_s0_load=True,
                          entry_cc=(t2s[1], t2f[1]))

    nc.compile()
    return nc


def kernel(x, edge_index, edge_weight, W1, b1, W2, b2):
    from concourse.bass_utils import run_bass_kernel_spmd

    idx_wr, w_rep, bidx_wr, xT, C, TOT = _host_prep(x, edge_index, edge_weight)
    W1n = np.asarray(W1, np.float32).astype(bf16)
    W2n = np.asarray(W2, np.float32)
    b1n = np.asarray(b1, np.float32).reshape(D_F, 1)
    b2n = np.asarray(b2, np.float32).reshape(D_F, 1)
    sel = np.zeros((128, D_F), np.float32)
    for g in range(NGROUP):
        for f in range(D_F):
            sel[16 * g + f, f] = 1.0
    id16 = np.eye(D_F, dtype=np.float32)

    nc = _build_program(C, TOT)

    in_maps = []
    for c in range(NC):
        m = {
            "xT": xT[c],
            "W1": W1n, "W2": W2n, "b1": b1n, "b2": b2n,
            "sel": sel, "id16": id16,
        }
        for S in range(NSH):
            m[f"idx{S}"] = idx_wr[S][c]
            m[f"w{S}"] = w_rep[S][c]
            m[f"bidx{S}"] = bidx_wr[S][c]
        in_maps.append(m)
    res = run_bass_kernel_spmd(nc, in_maps, list(range(NC)))
    out = np.zeros((N_NODES, D_F), np.float32)
    for c in range(NC):
        out[c * NPC_REAL:(c + 1) * NPC_REAL] = res.results[c]["out"][:NPC_REAL]
    return out


# revision 4
# speedup vs baseline: 13.0246x; 13.0246x over previous
"""2-layer GCN (gather/scatter message passing) on 8 trn2 NeuronCores — v2.

Strategy: nodes (and incoming edges) partitioned across 8 cores by dst
range. Each core computes x@W1 (bf16) for its slice; slices exchanged
via bf16 AllGather (split in halves to overlap with compute). Groups =
src octants: gpsimd core g gathers from the table of octant g (= core
g's h slice, [16 feats x 12800 nodes] f32, upcast from the bf16
AllGather), no table replication. Edge streams per (core, group) are
split by src half and dst chunk-half into 4 quarters, processed in
blocks of 4 dst-chunks (~5.6k idx) so each ap_gather amortizes its
table scan. Messages (DVE mult by bf16 weights, in place), prefix-sum
per block (DVE scan, in place), per-slot segment sums extracted by a
boundary ap_gather + diff; src-half partials accumulated via two
accumulating PE matmuls against a 0/1 selection matrix. Layer-2
log_softmax runs Exp/Ln in batched sweeps to avoid activation-table
thrash.
"""
import sys, os
sys.path.insert(0, '/opt/trn_rl_repo')

import numpy as np
import ml_dtypes

# ---- problem constants (hardcoded per contract) ----
N_NODES = 100000
N_EDGES = 6400000
D_IN, D_F = 512, 16
NC = 8                   # cores
NPC_REAL = 12500         # real nodes per core
DPC = 320                # dst slots per chunk
NCHUNK = 40              # chunks per core
NPC = DPC * NCHUNK       # padded nodes per core (12800)
NPAD = NPC * NC          # padded global nodes (102400)
NGROUP = 8               # 16-partition groups per core = src octants
HALF = NPC // 2          # src half size (6400)
NSH = 2                  # src halves
NCH = 2                  # dst chunk halves
CPH = NCHUNK // NCH      # chunks per dst half (20)
BLK = 4                  # dst chunks per gather block
NBLK = CPH // BLK        # blocks per quarter (5)
BND = 1 + BLK * DPC      # boundary entries per block (1281)
BNDP = ((BND + 15) // 16) * 16  # padded to 1296
PAC = 400                # phase-A column chunk (16 per half)
NPA = NPC // PAC         # 32 phase-A chunks

bf16 = ml_dtypes.bfloat16


def _pad_id(n):
    return (n // NPC_REAL) * NPC + (n % NPC_REAL)


def _host_prep(x, edge_index, edge_weight):
    """Per-core inputs + the shared per-(srchalf, chunk) length schedule."""
    src = np.asarray(edge_index[0], dtype=np.int64)
    dst = np.asarray(edge_index[1], dtype=np.int64)
    w = np.asarray(edge_weight, dtype=np.float32)

    spad = _pad_id(src)
    grp = spad // NPC                        # src octant 0..7
    loc = spad % NPC
    sh = loc // HALF                         # src half 0/1
    sidx = (loc % HALF).astype(np.int16)     # 0..6399
    dcore = dst // NPC_REAL
    dloc = dst % NPC_REAL
    chunk = dloc // DPC                      # 0..39
    dslot = dloc % DPC                       # 0..319

    order = np.lexsort((dslot, chunk, sh, grp, dcore))
    so_sidx = sidx[order]; so_w = w[order].astype(bf16)
    so_core = dcore[order]; so_grp = grp[order]
    so_sh = sh[order]; so_chunk = chunk[order]; so_slot = dslot[order]

    # counts per (core, group, srchalf, chunk)
    cgsk = ((so_core * NGROUP + so_grp) * NSH + so_sh) * NCHUNK + so_chunk
    counts = np.bincount(cgsk, minlength=NC * NGROUP * NSH * NCHUNK)
    counts4 = counts.reshape(NC, NGROUP, NSH, NCHUNK)
    # schedule per (srchalf, chunk): 1 pad + max count, rounded to 16
    # (16 keeps the idx partition-wrap and the %4 gather constraint)
    C = 1 + counts4.max(axis=(0, 1))         # [NSH, NCHUNK]
    C = ((C + 15) // 16) * 16
    C_off = np.concatenate([np.zeros((NSH, 1), np.int64), np.cumsum(C, axis=1)], axis=1)
    TOT = C_off[:, -1].astype(int)           # per src half

    # within-(core,group,sh,chunk) index of each sorted edge
    seg_off = np.concatenate([[0], np.cumsum(counts)])
    within = np.arange(N_EDGES) - seg_off[cgsk]

    idx_wr = []
    w_rep = []
    for S in range(NSH):
        m = so_sh == S
        TS = int(TOT[S])
        idx_all = np.zeros((NC, NGROUP, TS), dtype=np.int16)
        w_all = np.zeros((NC, NGROUP, TS), dtype=bf16)
        pos = C_off[S, so_chunk[m]] + 1 + within[m]
        lin = (so_core[m] * NGROUP + so_grp[m]) * TS + pos
        idx_all.reshape(-1)[lin] = so_sidx[m]
        w_all.reshape(-1)[lin] = so_w[m]
        # wrap idx into 16 partitions: part 16g+j holds list[j::16]
        iw = np.ascontiguousarray(
            idx_all.reshape(NC, NGROUP, TS // 16, 16).transpose(0, 1, 3, 2)
        ).reshape(NC, 128, TS // 16)
        idx_wr.append(iw)
        w_rep.append(np.repeat(w_all, 16, axis=1))

    # boundary position lists per (srchalf, block): [0] + 4 chunks x DPC ends
    cgsks = (((so_core * NGROUP + so_grp) * NSH + so_sh) * NCHUNK + so_chunk) * DPC + so_slot
    slot_counts = np.bincount(cgsks, minlength=NC * NGROUP * NSH * NCHUNK * DPC)
    slot_counts = slot_counts.reshape(NC, NGROUP, NSH, NCHUNK, DPC)
    bpos = np.cumsum(slot_counts, axis=4)    # end-of-slot counts

    bidx_wr = []
    for S in range(NSH):
        nblk_tot = NCH * NBLK                # 10 blocks
        blists = np.zeros((NC, NGROUP, nblk_tot, BNDP), dtype=np.int16)
        for H in range(NCH):
            for b in range(NBLK):
                k0 = H * CPH + b * BLK
                boff = C_off[S, k0]
                parts = [np.zeros((NC, NGROUP, 1), np.int64)]  # block-start pad
                for k in range(k0, k0 + BLK):
                    parts.append(C_off[S, k] - boff + bpos[:, :, S, k, :])
                lst = np.concatenate(parts, axis=2)            # [NC, NG, 1281]
                pad = np.repeat(lst[:, :, -1:], BNDP - BND, axis=2)
                blists[:, :, H * NBLK + b, :] = np.concatenate([lst, pad], axis=2)
        bw = np.ascontiguousarray(
            blists.reshape(NC, NGROUP, nblk_tot, BNDP // 16, 16)
            .transpose(0, 1, 4, 2, 3)
        ).reshape(NC, 128, nblk_tot * (BNDP // 16))
        bidx_wr.append(bw)

    # xT slices as [128, 4, NPC] bf16 (partition p holds rows p, 128+p,
    # 256+p, 384+p of x^T) so each phase-A chunk loads with ONE DMA
    xT = np.zeros((NC, 128, 4, NPC), dtype=bf16)
    xf = np.asarray(x, dtype=np.float32)
    for c in range(NC):
        xt = xf[c * NPC_REAL:(c + 1) * NPC_REAL, :].T.astype(bf16)  # [512, 12500]
        xT[c, :, :, :NPC_REAL] = xt.reshape(4, 128, NPC_REAL).transpose(1, 0, 2)

    return idx_wr, w_rep, bidx_wr, xT, C, TOT


def _build_program(C, TOT):
    import concourse.bass as bass
    import concourse.bacc as bacc
    import concourse.mybir as mybir
    from concourse.tile import TileContext

    f32 = mybir.dt.float32
    bf = mybir.dt.bfloat16
    i16 = mybir.dt.int16
    AO = mybir.AluOpType
    AF = mybir.ActivationFunctionType
    C_off = np.concatenate([np.zeros((NSH, 1), np.int64), np.cumsum(C, axis=1)], axis=1)

    nc = bacc.Bacc("TRN2", target_bir_lowering=False, debug=False, num_devices=NC)
    from concourse.bass import BassGpSimd

    def collective_on(eng, **kw):
        """All collectives on gpsimd (the only engine neuronxcc accepts
        here). Issue positions are chosen so the Pool-queue park during the
        transfer lands where DVE has a backlog to drain."""
        return nc.gpsimd.collective_compute(
            "AllGather", mybir.AluOpType.bypass,
            replica_groups=[list(range(NC))], **kw)

    # inputs
    xT_d = nc.dram_tensor("xT", [128, 4, NPC], bf, kind="ExternalInput")
    idx_d = [nc.dram_tensor(f"idx{S}", [128, int(TOT[S]) // 16], i16, kind="ExternalInput")
             for S in range(NSH)]
    w_d = [nc.dram_tensor(f"w{S}", [128, int(TOT[S])], bf, kind="ExternalInput")
           for S in range(NSH)]
    bidx_d = [nc.dram_tensor(f"bidx{S}", [128, NCH * NBLK * (BNDP // 16)], i16,
                             kind="ExternalInput") for S in range(NSH)]
    W1_d = nc.dram_tensor("W1", [D_IN, D_F], bf, kind="ExternalInput")
    W2_d = nc.dram_tensor("W2", [D_F, D_F], f32, kind="ExternalInput")
    b1_d = nc.dram_tensor("b1", [D_F, 1], f32, kind="ExternalInput")
    b2_d = nc.dram_tensor("b2", [D_F, 1], f32, kind="ExternalInput")
    sel_d = nc.dram_tensor("sel", [128, D_F], f32, kind="ExternalInput")
    id16_d = nc.dram_tensor("id16", [D_F, D_F], f32, kind="ExternalInput")
    out_d = nc.dram_tensor("out", [NPC, D_F], f32, kind="ExternalOutput")

    # internal DRAM: per-half h slices + allgathered tables (bf16)
    t1s = [nc.dram_tensor(f"t1s{S}", [D_F, HALF], bf) for S in range(NSH)]
    t2s = [nc.dram_tensor(f"t2s{S}", [D_F, HALF], bf) for S in range(NSH)]
    t1f = [nc.dram_tensor(f"t1f{S}", [NC * D_F, HALF], bf, addr_space="Shared")
           for S in range(NSH)]
    t2f = [nc.dram_tensor(f"t2f{S}", [NC * D_F, HALF], bf, addr_space="Shared")
           for S in range(NSH)]

    with TileContext(nc) as tc:
        with (tc.tile_pool(name="const", bufs=1) as cpool,
              tc.tile_pool(name="pA", bufs=4) as pa,
              tc.tile_pool(name="pAp", bufs=2, space="PSUM") as pap,
              tc.tile_pool(name="tblp", bufs=1) as tp,
              tc.tile_pool(name="tbfp", bufs=2) as tbp,
              tc.tile_pool(name="ed", bufs=2) as ep,
              tc.tile_pool(name="dva", bufs=1) as dap,
              tc.tile_pool(name="zs", bufs=1) as zp,
              tc.tile_pool(name="l2", bufs=2) as l2p,
              tc.tile_pool(name="aggp", bufs=2, space="PSUM") as ap_,
              tc.tile_pool(name="psz", bufs=2, space="PSUM") as pzp,
              tc.tile_pool(name="pst", bufs=2, space="PSUM") as ptp):
            w1t = cpool.tile([128, 4, D_F], bf)
            for kp in range(4):
                nc.sync.dma_start(out=w1t[:, kp, :], in_=W1_d[kp * 128:(kp + 1) * 128, :])
            w2t = cpool.tile([D_F, D_F], f32)
            nc.sync.dma_start(out=w2t[:], in_=W2_d[:])
            b1t = cpool.tile([D_F, 1], f32)
            nc.sync.dma_start(out=b1t[:], in_=b1_d[:])
            b2t = cpool.tile([D_F, 1], f32)
            nc.sync.dma_start(out=b2t[:], in_=b2_d[:])
            selt = cpool.tile([128, D_F], f32)
            nc.sync.dma_start(out=selt[:], in_=sel_d[:])
            id16t = cpool.tile([D_F, D_F], f32)
            nc.sync.dma_start(out=id16t[:], in_=id16_d[:])

            # ---------------- phase A: t1 = W1^T @ xT (bf16) --------------
            for ca in range(NPA):
                c0 = ca * PAC
                ps = pap.tile([D_F, PAC], f32, tag="t1ps")
                xtw = pa.tile([128, 4, PAC], bf, tag="xt")
                nc.sync.dma_start(out=xtw[:], in_=xT_d[:, :, c0:c0 + PAC])
                for kp in range(4):
                    nc.tensor.matmul(ps[:], lhsT=w1t[:, kp, :], rhs=xtw[:, kp, :],
                                     start=(kp == 0), stop=(kp == 3))
                if ca % 2 == 0:
                    t1c = pa.tile([D_F, 2, PAC], bf, tag="t1c", bufs=2)
                nc.vector.tensor_copy(t1c[:, ca % 2, :], ps[:])
                if ca % 2 == 1:
                    S = c0 // HALF
                    cs = (ca - 1) * PAC - S * HALF
                    nc.sync.dma_start(out=t1s[S][:, cs:cs + 2 * PAC], in_=t1c[:])
                if ca == NPA // 2 - 1:
                    collective_on(nc.sync, ins=[t1s[0][:]], outs=[t1f[0][:]])
            if True:
                tblh = [tp.tile([128, HALF], f32, name=f"tblh{S}", tag=f"tbl{S}")
                        for S in range(NSH)]
                dva = dap.tile([128, CPH * DPC], f32)
                zsave = zp.tile([128, 3 * CPH, D_F], f32)
                smsave = zp.tile([128, 3 * CPH], f32)

                def load_table(tf, S):
                    """upcast bf16 allgathered half into tblh[S].

                    The staging DMA is issued from the Act queue: it waits on
                    the AllGather, and SP's in-order queue must not head-block
                    the edge-stream prefetch DMAs behind it."""
                    for piece in range(2):
                        pc0 = piece * (HALF // 2)
                        tbf = tbp.tile([128, HALF // 2], bf, tag="tbf")
                        nc.scalar.dma_start(out=tbf[:], in_=tf[:, pc0:pc0 + HALF // 2])
                        nc.scalar.activation(tblh[S][:, pc0:pc0 + HALF // 2],
                                             tbf[:], AF.Copy)

                def block_front(S, H, b):
                    """loads + gather + mult + scan for one block"""
                    k0 = H * CPH + b * BLK
                    o0 = int(C_off[S, k0]); o1 = int(C_off[S, k0 + BLK])
                    B = o1 - o0
                    idxt = ep.tile([128, B // 16], i16, tag="idxt")
                    nc.sync.dma_start(out=idxt[:], in_=idx_d[S][:, o0 // 16:o1 // 16])
                    wt = ep.tile([128, B], bf, tag="wt")
                    nc.sync.dma_start(out=wt[:], in_=w_d[S][:, o0:o1])
                    bb = (H * NBLK + b) * (BNDP // 16)
                    bit = ep.tile([128, BNDP // 16], i16, tag="bit")
                    nc.sync.dma_start(out=bit[:], in_=bidx_d[S][:, bb:bb + BNDP // 16])

                    gt = ep.tile([128, B], f32, tag="gt")
                    nc.gpsimd.ap_gather(gt[:], tblh[S][:], idxt[:], channels=128,
                                        num_elems=HALF, d=1, num_idxs=B)
                    nc.vector.tensor_tensor(out=gt[:], in0=gt[:], in1=wt[:], op=AO.mult)
                    nc.vector.tensor_tensor_scan(gt[:], gt[:], gt[:], 0.0,
                                                 AO.add, AO.bypass)
                    return gt, bit, B

                def block_back(st, S, H, b):
                    """boundary gather + diff (Pool) for a finished block"""
                    gt, bit, B = st
                    bv = ep.tile([128, BNDP], f32, tag="bv")
                    nc.gpsimd.ap_gather(bv[:], gt[:], bit[:], channels=128,
                                        num_elems=B, d=1, num_idxs=BNDP)
                    dcols = BLK * DPC
                    if S == 0:
                        nc.gpsimd.tensor_tensor(
                            out=dva[:, b * dcols:(b + 1) * dcols],
                            in0=bv[:, 1:BND], in1=bv[:, 0:BND - 1], op=AO.subtract)
                    else:
                        dv = ep.tile([128, dcols], f32, tag="dv")
                        nc.gpsimd.tensor_tensor(
                            out=dv[:], in0=bv[:, 1:BND], in1=bv[:, 0:BND - 1],
                            op=AO.subtract)
                        nc.gpsimd.tensor_tensor(
                            out=dva[:, b * dcols:(b + 1) * dcols],
                            in0=dva[:, b * dcols:(b + 1) * dcols],
                            in1=dv[:], op=AO.add)

                def epi_chunk(kk, H, layer):
                    agg = ap_.tile([D_F, DPC], f32, tag="agg")
                    nc.tensor.matmul(agg[:], lhsT=selt[:],
                                     rhs=dva[:, kk * DPC:(kk + 1) * DPC],
                                     start=True, stop=True)
                    if layer == 1:
                        hc = ep.tile([D_F, DPC], bf, tag="hc")
                        nc.scalar.activation(hc[:], agg[:], AF.Relu, bias=b1t[:])
                        k = H * CPH + kk
                        Sd = k * DPC // HALF
                        cd = k * DPC - Sd * HALF
                        nc.sync.dma_start(out=t2s[Sd][:, cd:cd + DPC], in_=hc[:])
                    else:
                        asb = l2p.tile([D_F, DPC], f32, tag="asb", bufs=3)
                        nc.scalar.activation(asb[:], agg[:], AF.Copy)
                        zps = pzp.tile([D_F, DPC], f32, tag="zps")
                        nc.tensor.matmul(zps[:], lhsT=w2t[:], rhs=asb[:],
                                         start=True, stop=True)
                        zsb = l2p.tile([D_F, DPC], f32, tag="zsb", bufs=3)
                        nc.scalar.activation(zsb[:], zps[:], AF.Identity, bias=b2t[:])
                        for ji, j0 in enumerate(range(0, DPC, 128)):
                            bw = min(128, DPC - j0)
                            t = 3 * kk + ji
                            tps = ptp.tile([128, D_F], f32, tag="tps")
                            nc.tensor.transpose(tps[:bw, :], zsb[:, j0:j0 + bw],
                                                id16t[:])
                            mx = l2p.tile([128, 1], f32, tag="mx", bufs=4)
                            nc.vector.reduce_max(mx[:bw, :], tps[:bw, :],
                                                 axis=mybir.AxisListType.X)
                            nc.vector.tensor_scalar(out=zsave[:bw, t, :],
                                                    in0=tps[:bw, :],
                                                    scalar1=mx[:bw, :], scalar2=None,
                                                    op0=AO.subtract)
                            ez = l2p.tile([128, D_F], f32, tag="ez", bufs=4)
                            nc.scalar.activation(ez[:bw, :], zsave[:bw, t, :], AF.Exp,
                                                 accum_out=smsave[:bw, t:t + 1])

                lsT = [None]

                def final_sweep(H, k0, k1):
                    if k0 == 0:
                        lsT[0] = zp.tile([128, 3 * CPH], f32, name="lst", tag="ls")
                    ls = lsT[0]
                    nc.scalar.activation(ls[:, 3 * k0:3 * k1],
                                         smsave[:, 3 * k0:3 * k1], AF.Ln)
                    for kk in range(k0, k1):
                        for ji, j0 in enumerate(range(0, DPC, 128)):
                            bw = min(128, DPC - j0)
                            t = 3 * kk + ji
                            ot = l2p.tile([128, D_F], f32, tag="ot", bufs=8)
                            nc.vector.tensor_scalar(out=ot[:bw, :],
                                                    in0=zsave[:bw, t, :],
                                                    scalar1=ls[:bw, t:t + 1],
                                                    scalar2=None,
                                                    op0=AO.subtract)
                            r0 = (H * CPH + kk) * DPC + j0
                            nc.sync.dma_start(out=out_d[r0:r0 + bw, :],
                                              in_=ot[:bw, :])

                def aggregate(tfs, layer, next_tfs=None, skip_s0_load=False,
                              entry_cc=None, mid_cc=None):
                    if not skip_s0_load:
                        load_table(tfs[0], 0)
                    if layer == 2:
                        nc.vector.memset(smsave[:], 1.0)
                    blocks = [(H, S, b) for H in range(NCH) for S in range(NSH)
                              for b in range(NBLK)]

                    def post_back(H, S, b):
                        """work enabled by block (H,S,b)'s dva contribution"""
                        if S != 1:
                            return
                        for kk in range(b * BLK, (b + 1) * BLK):
                            epi_chunk(kk, H, layer)
                        if layer == 2 and b == NBLK - 2:
                            # chunks 0..4*(NBLK-1)-1 of this half are done
                            final_sweep(H, 0, BLK * (NBLK - 1) - BLK)
                        if layer == 2 and b == NBLK - 1:
                            final_sweep(H, BLK * (NBLK - 1) - BLK, CPH)
                            if H == 0:
                                nc.vector.memset(smsave[:], 1.0)

                    prev = None
                    for (H, S, b) in blocks:
                        if (H, S, b) == (0, 1, 0):
                            if entry_cc is not None:
                                collective_on(nc.gpsimd, ins=[entry_cc[0][:]],
                                              outs=[entry_cc[1][:]])
                            load_table(tfs[1], 1)
                        if (H, S, b) == (1, 1, 0):
                            if mid_cc is not None:
                                collective_on(nc.gpsimd, ins=[mid_cc[0][:]],
                                              outs=[mid_cc[1][:]])
                            if next_tfs is not None:
                                # prefetch next pass's S0 table (waits the
                                # mid collective + WAR on this pass's S0 reads)
                                load_table(next_tfs[0], 0)
                        st = block_front(S, H, b)
                        if prev is not None:
                            block_back(st_prev, *reversed_args(prev))
                            post_back(*prev)
                        prev = (H, S, b); st_prev = st
                    block_back(st_prev, *reversed_args(prev))
                    post_back(*prev)

                def reversed_args(hsb):
                    H, S, b = hsb
                    return (S, H, b)

                aggregate(t1f, 1, next_tfs=t2f,
                          entry_cc=(t1s[1], t1f[1]), mid_cc=(t2s[0], t2f[0]))
                aggregate(t2f, 2, skip_s0_load=True,
                          entry_cc=(t2s[1], t2f[1]))

    nc.compile()
    return nc


def kernel(x, edge_index, edge_weight, W1, b1, W2, b2):
    from concourse.bass_utils import run_bass_kernel_spmd

    idx_wr, w_rep, bidx_wr, xT, C, TOT = _host_prep(x, edge_index, edge_weight)
    W1n = np.asarray(W1, np.float32).astype(bf16)
    W2n = np.asarray(W2, np.float32)
    b1n = np.asarray(b1, np.float32).reshape(D_F, 1)
    b2n = np.asarray(b2, np.float32).reshape(D_F, 1)
    sel = np.zeros((128, D_F), np.float32)
    for g in range(NGROUP):
        for f in range(D_F):
            sel[16 * g + f, f] = 1.0
    id16 = np.eye(D_F, dtype=np.float32)

    nc = _build_program(C, TOT)

    in_maps = []
    for c in range(NC):
        m = {
            "xT": xT[c],
            "W1": W1n, "W2": W2n, "b1": b1n, "b2": b2n,
            "sel": sel, "id16": id16,
        }
        for S in range(NSH):
            m[f"idx{S}"] = idx_wr[S][c]
            m[f"w{S}"] = w_rep[S][c]
            m[f"bidx{S}"] = bidx_wr[S][c]
        in_maps.append(m)
    res = run_bass_kernel_spmd(nc, in_maps, list(range(NC)))
    out = np.zeros((N_NODES, D_F), np.float32)
    for c in range(NC):
        out[c * NPC_REAL:(c + 1) * NPC_REAL] = res.results[c]["out"][:NPC_REAL]
    return out
